# revision 1
# baseline (speedup 1.0000x reference)
"""Trainium2 Bass kernel for a 2-layer GAT (nn_GAT_781684048444).

Strategy (8 NeuronCores, SPMD):
  - Nodes are assigned to 80 windows (8 cores x 10 windows x 128 slots) by a
    greedy in-degree balancer so every window owns <= 2048 incoming edges.
    This is a pure host-side renumbering; gather tables stay in original
    node-id space and the final output is inverse-permuted.
  - Edges are grouped by their dst window and padded to K 128-edge tiles per
    window (K=16 when balanced), so one static program serves all cores.
  - Stage 0 (replicated on every core): two matmuls per 128-node block build
    a packed table row [512 bf16 msg (head-minor) | 8 fp32 s_src | 8 fp32
    s_dst | pad] = 1280B: xh = x @ W1 in bf16 plus the folded attention dot
    products [s_src | s_dst] = x @ [W1.a_src | W1.a_dst] in fp32.
  - Layer-1 edge phase: dma_gather full packed rows by src and the fp32 alpha
    columns by dst; alpha = leakyrelu(s_src+s_dst+ew*c1) in fp32; exp -> bf16;
    scale messages (head-minor layout keeps every DVE operand stride-1 so the
    broadcast multiply runs in the fast mode); scatter-add via one-hot bf16
    selT matmuls accumulated in fp32 PSUM. h1 = elu(U / D) (segment softmax
    denominator folded in afterwards). The layer-2 projection h2 = h1 @ W_aug2
    is interleaved per window so the collective can start immediately.
  - Layer 2: h2_aug = h1 @ W_aug2 per core -> compact [1280, 8] fp32 table ->
    AllGather of just 320KB -> expand to 256B rows locally -> same
    gather/attend/scatter with the selT tiles kept resident from layer 1.
    dst-side scalars gather from the LOCAL table so they overlap the
    collective.
"""

import os
import sys

import ml_dtypes
import numpy as np

sys.path.insert(0, "/opt/trn_rl_repo")

from concourse import bacc, bass, mybir, tile  # noqa: E402
from concourse.bass import AP  # noqa: E402
from concourse.bass_utils import run_bass_kernel_spmd  # noqa: E402

N, E = 10000, 160000
IN, HID, OUT, H = 128, 64, 4, 8
C1 = H * HID  # 512 layer-1 out width
TMW = 320     # packed msg-table row, fp32 units: 256=512bf16 msg, 8 s_src, 8 s_dst, 48 pad
TAW = 64      # alpha columns width fp32 (8 s_src + 8 s_dst + 48 pad)
T2W = 64      # expanded table2 row width (4 h2 + 1 s_src2 + 1 s_dst2 + pad)
NCORES = 8
NPC = N // NCORES          # 1250 nodes per core
WIN = 128                  # window = 128 node slots
NB = 10                    # windows per core
NPAD = NB * WIN            # 1280 node slots per core
NWIN = NCORES * NB         # 80 windows total
NBLK = (N + 127) // 128    # 79 stage-0 node blocks
MAXI = 1024                # dma_gather num_idxs hardware ring limit

FP = mybir.dt.float32
BF = mybir.dt.bfloat16
I16 = mybir.dt.int16

_CACHE = {}

LAST_EXEC_NS = None
LAST_RESULTS = None


def _wrap_idx(vals):
    """int16 gather index layout: idx i -> [i%16, i//16], tiled to 128 partitions."""
    n = vals.shape[0]
    w = np.zeros((16, n // 16), np.int16)
    w[np.arange(n) % 16, np.arange(n) // 16] = vals.astype(np.int16)
    return np.tile(w, (8, 1))


def _build_program(KC, NCH):
    K = KC * NCH           # tiles per window
    SL = KC * 128          # edge slots per chunk
    PHASE = int(os.environ.get("BASS_GAT_PHASE", "3"))

    nc = bacc.Bacc("TRN2", target_bir_lowering=False, debug=False, num_devices=NCORES)

    # ---- DRAM parameters (replicated across cores unless noted) ----
    xTf_d = nc.dram_tensor("xTf", [IN, N], FP, kind="ExternalInput")
    xTb_d = nc.dram_tensor("xTb", [IN, N], BF, kind="ExternalInput")
    W1b_d = nc.dram_tensor("W1b", [IN, C1], BF, kind="ExternalInput")
    Wsd_d = nc.dram_tensor("Wsd", [IN, TAW], FP, kind="ExternalInput")
    W2_d = nc.dram_tensor("W2r", [128, 4, 8], FP, kind="ExternalInput")
    c1_d = nc.dram_tensor("c1rep", [128, H], FP, kind="ExternalInput")
    b1_d = nc.dram_tensor("b1rep", [128, C1], FP, kind="ExternalInput")
    b2_d = nc.dram_tensor("b2rep", [128, 8], FP, kind="ExternalInput")
    ident_d = nc.dram_tensor("ident", [128, 128], FP, kind="ExternalInput")
    # per-core edge data
    srcg_d = nc.dram_tensor("srcg", [128, NB, NCH, KC * 8], I16, kind="ExternalInput")
    dstg_d = nc.dram_tensor("dstg", [128, NB, NCH, KC * 8], I16, kind="ExternalInput")
    srcg2_d = nc.dram_tensor("srcg2", [128, NB, NCH, KC * 8], I16, kind="ExternalInput")
    dstg2_d = nc.dram_tensor("dstg2", [128, NB, NCH, KC * 8], I16, kind="ExternalInput")
    ew_d = nc.dram_tensor("ew", [128, NB, NCH, KC], FP, kind="ExternalInput")
    selT_d = nc.dram_tensor("selT", [128, NB, NCH, SL], BF, kind="ExternalInput")

    out_d = nc.dram_tensor("out_own", [NPAD, 4], FP, kind="ExternalOutput")

    # ---- internal DRAM ----
    tableM = nc.dram_tensor("tableM", [NBLK * 128, TMW], FP)  # packed bf16 msg + fp32 alpha
    table2x = nc.dram_tensor("table2x", [NPAD, T2W], FP)      # local expanded
    table2c = nc.dram_tensor("table2c", [NPAD, 8], FP)        # compact AG input
    table2cf = nc.dram_tensor("table2cf", [NCORES * NPAD, 8], FP, addr_space="Shared")
    table2f = nc.dram_tensor("table2f", [NCORES * NPAD, T2W], FP)  # global expanded

    c2_host = _build_program.c2_host  # python float baked into instructions

    XCH = 10  # stage-0 x column chunks of 10 blocks

    with tile.TileContext(nc) as tc:
        with (
            tc.tile_pool(name="const", bufs=1) as constp,
            tc.tile_pool(name="idx", bufs=1) as idxp,
            tc.tile_pool(name="h1p", bufs=1) as h1p,
            tc.tile_pool(name="selp", bufs=1) as selp,
        ):
            W1b = constp.tile([IN, C1], BF)
            nc.scalar.dma_start(W1b[:], W1b_d[:])
            Wsd = constp.tile([IN, TAW], FP)
            nc.scalar.dma_start(Wsd[:], Wsd_d[:])
            W2sb = constp.tile([128, 4, 8], FP)
            nc.scalar.dma_start(W2sb[:], W2_d[:])
            c1rep = constp.tile([128, H], FP)
            nc.scalar.dma_start(c1rep[:], c1_d[:])
            b1rep = constp.tile([128, C1], FP)
            nc.scalar.dma_start(b1rep[:], b1_d[:])
            b2rep = constp.tile([128, 8], FP)
            nc.scalar.dma_start(b2rep[:], b2_d[:])
            ident = constp.tile([128, 128], FP)
            nc.scalar.dma_start(ident[:], ident_d[:])
            ewsb = constp.tile([128, NB, NCH, KC], FP)
            nc.scalar.dma_start(ewsb[:], ew_d[:])

            srcg = idxp.tile([128, NB, NCH, KC * 8], I16)
            dstg = idxp.tile([128, NB, NCH, KC * 8], I16)
            srcg2 = idxp.tile([128, NB, NCH, KC * 8], I16)
            dstg2 = idxp.tile([128, NB, NCH, KC * 8], I16)
            # all selection tiles resident for both layers (bf16)
            selsb = selp.tile([128, NB, NCH, SL], BF)

            h1own = h1p.tile([128, NB, C1], FP)

            # ========== stage 0: tableM (bf16) + tableA (fp32) ==========
            with (
                tc.tile_pool(name="s0", bufs=3) as s0p,
                tc.tile_pool(name="s0x", bufs=8) as s0xp,
                tc.tile_pool(name="s0ps", bufs=3, space="PSUM") as s0ps,
                tc.tile_pool(name="s0z", bufs=1) as s0zp,
            ):
                xfch, xbch = [], []
                for xc in range((NBLK + XCH - 1) // XCH):
                    c0 = xc * XCH * 128
                    cz = min(N, (xc + 1) * XCH * 128)
                    xf = s0xp.tile([IN, XCH * 128], FP, tag="xf")
                    nc.sync.dma_start(xf[:, : cz - c0], xTf_d[:, c0:cz])
                    xfch.append(xf)
                    xb = s0xp.tile([IN, XCH * 128], BF, tag="xb")
                    nc.sync.dma_start(xb[:, : cz - c0], xTb_d[:, c0:cz])
                    xbch.append(xb)
                # edge-phase metadata loads queue behind the x chunks on SP
                nc.sync.dma_start(srcg[:], srcg_d[:])
                nc.sync.dma_start(dstg[:], dstg_d[:])
                nc.sync.dma_start(srcg2[:], srcg2_d[:])
                nc.sync.dma_start(dstg2[:], dstg2_d[:])
                nc.sync.dma_start(selsb[:], selT_d[:])
                # pre-zero table2f (one contiguous write) so the post-collective
                # expansion only has to fill cols 0:8
                nrow = NCORES * NPAD
                J = nrow // 128
                ztf = s0zp.tile([128, (J * T2W) // 4], FP, tag="ztf")
                nc.vector.memset(ztf[:], 0.0)
                for zk in range(4):
                    ztf_out = AP(table2f[:].tensor, zk * (J * T2W) // 4,
                                 [(J * T2W, 128), (1, (J * T2W) // 4)])
                    nc.scalar.dma_start(ztf_out, ztf[:])
                # zero the row padding past N so gather-source views stay finite
                if NBLK * 128 > N:
                    zpad = s0p.tile([128, TMW], FP, tag="zpad")
                    nc.vector.memset(zpad[:], 0.0)
                    npad_rows = NBLK * 128 - N
                    nc.gpsimd.dma_start(tableM[N : NBLK * 128, :], zpad[:npad_rows, :])
                NFULL = (NBLK // 4) * 4  # blocks batched in full 4-block groups
                stgM = None
                for b in range(NBLK):
                    rows = min(128, N - b * 128)
                    off = (b % XCH) * 128
                    psA = s0ps.tile([128, 512], FP, tag="psA")
                    psB = s0ps.tile([128, TAW], FP, tag="psB")
                    nc.tensor.matmul(psA[:rows, :], xbch[b // XCH][:, off : off + rows],
                                     W1b[:], start=True, stop=True)
                    nc.tensor.matmul(psB[:rows, :], xfch[b // XCH][:, off : off + rows],
                                     Wsd[:], start=True, stop=True)
                    if b < NFULL:
                        if b % 4 == 0:
                            stgM = s0p.tile([128, 4, TMW], FP, tag="stgM")
                        bi = b % 4
                        if b % 3 != 2:
                            nc.vector.tensor_copy(stgM[:, bi, 0:256].bitcast(BF), psA[:, :])
                        else:
                            nc.scalar.copy(stgM[:, bi, 0:256].bitcast(BF), psA[:, :])
                        nc.vector.tensor_copy(stgM[:, bi, 256:320], psB[:, :])
                        if b % 4 == 3:
                            b0 = (b // 4) * 4
                            outM = AP(tableM[:].tensor, b0 * 128 * TMW,
                                      [(TMW, 128), (128 * TMW, 4), (1, TMW)])
                            nc.gpsimd.dma_start(outM, stgM[:, 0:4, :])
                    else:
                        sM = s0p.tile([128, TMW], FP, tag="stgM1")
                        nc.vector.tensor_copy(sM[:rows, 0:256].bitcast(BF), psA[:rows, :])
                        nc.vector.tensor_copy(sM[:rows, 256:320], psB[:rows, :])
                        nc.gpsimd.dma_start(tableM[b * 128 : b * 128 + rows, :], sM[:rows, :])

            # ================= layer 1 edge phase =================
            if PHASE >= 1:
                with (
                    tc.tile_pool(name="g1", bufs=3) as g1p,
                    tc.tile_pool(name="gd1", bufs=3) as gd1p,
                    tc.tile_pool(name="al1", bufs=3) as al1p,
                    tc.tile_pool(name="wend", bufs=2) as wendp,
                    tc.tile_pool(name="l2h", bufs=3) as l2hp,
                    tc.tile_pool(name="ps1", bufs=2, space="PSUM") as ps1p,
                    tc.tile_pool(name="l2ps", bufs=2, space="PSUM") as l2ps,
                    tc.tile_pool(name="l2tp", bufs=2, space="PSUM") as l2tp,
                ):
                    for w in range(NB):
                        psU = ps1p.tile([128, 512], FP)
                        psD = ps1p.tile([128, 8], FP)
                        for ch in range(NCH):
                            g = g1p.tile([128, KC, TMW], FP)
                            nc.gpsimd.dma_gather(
                                g[:], tableM[:], srcg[:, w, ch, :], SL, SL, TMW
                            )
                            gd = gd1p.tile([128, KC, TAW], FP)
                            nc.gpsimd.dma_gather(
                                gd[:], tableM[:, 256:320], dstg[:, w, ch, :], SL, SL, TAW,
                                elem_step=TMW,
                            )
                            # alpha = s_src[src] + s_dst[dst] + ew*c1 -> [128, KC, 8]
                            a = al1p.tile([128, KC, 8], FP)
                            nc.vector.tensor_tensor(
                                out=a[:], in0=g[:, :, 256:264], in1=gd[:, :, 8:16],
                                op=mybir.AluOpType.add,
                            )
                            ewc = al1p.tile([128, KC, 8], FP)
                            ew_b = ewsb[:, w, ch, :].to_broadcast([128, KC, 8])
                            c1_b = AP(c1rep[:].tensor, c1rep[:].offset,
                                      [c1rep[:].ap[0], (0, KC), c1rep[:].ap[1]])
                            nc.vector.tensor_tensor(out=ewc[:], in0=ew_b, in1=c1_b,
                                                    op=mybir.AluOpType.mult)
                            nc.vector.tensor_tensor(out=a[:], in0=a[:], in1=ewc[:],
                                                    op=mybir.AluOpType.add)
                            # leaky relu: max(0.2*a, a), then exp -> bf16
                            nc.vector.scalar_tensor_tensor(
                                out=a[:], in0=a[:], scalar=0.2, in1=a[:],
                                op0=mybir.AluOpType.mult, op1=mybir.AluOpType.max)
                            ahb = al1p.tile([128, KC, 1, 8], BF)
                            nc.scalar.activation(ahb[:, :, 0, :], a[:],
                                                 mybir.ActivationFunctionType.Exp)
                            # msg *= alpha-hat. Messages are stored head-minor
                            # (col = c*8+h), so the broadcast lands on the
                            # middle dim and every operand keeps a stride-1
                            # last dim -> TensorTensor runs in its 2x_1p mode.
                            mv = g[:, :, 0:256].bitcast(BF)
                            msg4 = mv.rearrange("p t (c h) -> p t c h", h=8)
                            ah4 = ahb[:].to_broadcast([128, KC, 64, 8])
                            nc.vector.tensor_tensor(out=msg4, in0=msg4, in1=ah4,
                                                    op=mybir.AluOpType.mult)
                            for t in range(KC):
                                ti = ch * KC + t
                                st = ti == 0
                                sp = ti == K - 1
                                sel = selsb[:, w, ch, t * 128 : (t + 1) * 128]
                                nc.tensor.matmul(psU[:], sel, g[:, t, 0:256].bitcast(BF),
                                                 start=st, stop=sp)
                                nc.tensor.matmul(psD[:], sel, ahb[:, t, 0, :],
                                                 start=st, stop=sp)
                        # ---- window finalize: h1 = elu(U/D + b1) ----
                        dpe = wendp.tile([128, 8], FP)
                        nc.vector.tensor_scalar_add(dpe[:], psD[:], 1e-16)
                        dr = wendp.tile([128, 1, 8], FP)
                        nc.vector.reciprocal(dr[:, 0, :], dpe[:])
                        h1v = h1own[:, w, :]
                        h1v3 = h1v.rearrange("p (c h) -> p c h", h=8)
                        psU3 = psU[:].rearrange("p (c h) -> p c h", h=8)
                        nc.vector.tensor_tensor(out=h1v3, in0=psU3,
                                                in1=dr[:].to_broadcast([128, 64, 8]),
                                                op=mybir.AluOpType.mult)
                        if _build_program.use_b1:
                            nc.vector.tensor_tensor(out=h1v, in0=h1v, in1=b1rep[:],
                                                    op=mybir.AluOpType.add)
                        tmin = wendp.tile([128, C1], FP)
                        nc.vector.tensor_scalar_min(tmin[:], h1v, 0.0)
                        nc.scalar.activation(tmin[:], tmin[:],
                                             mybir.ActivationFunctionType.Exp)
                        nc.vector.tensor_scalar_max(h1v, h1v, 0.0)
                        nc.vector.scalar_tensor_tensor(
                            out=h1v, in0=h1v, scalar=-1.0, in1=tmin[:],
                            op0=mybir.AluOpType.add, op1=mybir.AluOpType.add,
                        )
                        # ---- layer-2 projection for this window, interleaved
                        if PHASE >= 2:
                            ps2 = l2ps.tile([128, 8], FP)
                            for kc in range(4):
                                tps = l2tp.tile([128, 128], FP)
                                nc.tensor.transpose(
                                    tps[:], h1own[:, w, kc * 128 : (kc + 1) * 128],
                                    ident[:])
                                tsb = l2hp.tile([128, 128], FP)
                                nc.scalar.copy(tsb[:], tps[:])
                                nc.tensor.matmul(ps2[:], tsb[:], W2sb[:, kc, :],
                                                 start=(kc == 0), stop=(kc == 3))
                            st2 = l2hp.tile([128, T2W], FP)
                            nc.vector.memset(st2[:, 8:T2W], 0.0)
                            nc.scalar.copy(st2[:, 0:8], ps2[:])
                            nc.sync.dma_start(table2c[w * 128 : (w + 1) * 128, :],
                                              st2[:, 0:8])
                            nc.sync.dma_start(table2x[w * 128 : (w + 1) * 128, :],
                                              st2[:])

            # ========== layer 2 exchange: AllGather + expansion ==========
            if PHASE >= 2:
                    nc.gpsimd.collective_compute(
                        "AllGather",
                        mybir.AluOpType.bypass,
                        replica_groups=[list(range(NCORES))],
                        ins=[table2c[:]],
                        outs=[table2cf[:]],
                    )
                    # drop the AG result into cols 0:8 of the pre-zeroed
                    # 256B-row table with one DRAM->DRAM strided copy
                    nc.sync.dma_start(table2f[:, 0:8], table2cf[:])

            # ================= layer 2 edge phase =================
            if PHASE >= 3:
                with (
                    tc.tile_pool(name="g2", bufs=3) as g2p,
                    tc.tile_pool(name="gd2", bufs=NB * NCH) as gd2p,
                    tc.tile_pool(name="al2", bufs=3) as al2p,
                    tc.tile_pool(name="wend2", bufs=2) as wend2p,
                    tc.tile_pool(name="ps2", bufs=2, space="PSUM") as ps2p,
                ):
                    gds = []
                    for w in range(NB):
                        for ch in range(NCH):
                            gd = gd2p.tile([128, KC, T2W], FP, tag="gd")
                            nc.gpsimd.dma_gather(
                                gd[:], table2x[:], dstg2[:, w, ch, :], SL, SL, T2W
                            )
                            gds.append(gd)
                    for w in range(NB):
                        psO = ps2p.tile([128, 8], FP)
                        for ch in range(NCH):
                            gd = gds[w * NCH + ch]
                            gs = g2p.tile([128, KC, T2W], FP)
                            nc.gpsimd.dma_gather(
                                gs[:], table2f[:], srcg2[:, w, ch, :], SL, SL, T2W
                            )

                            a2 = al2p.tile([128, KC, 1], FP)
                            nc.vector.tensor_tensor(out=a2[:], in0=gs[:, :, 4:5],
                                                    in1=gd[:, :, 5:6],
                                                    op=mybir.AluOpType.add)
                            ew_b = ewsb[:, w, ch, :].to_broadcast([128, KC, 1])
                            nc.vector.scalar_tensor_tensor(
                                out=a2[:], in0=ew_b, scalar=float(c2_host), in1=a2[:],
                                op0=mybir.AluOpType.mult, op1=mybir.AluOpType.add,
                            )
                            nc.vector.scalar_tensor_tensor(
                                out=a2[:], in0=a2[:], scalar=0.2, in1=a2[:],
                                op0=mybir.AluOpType.mult, op1=mybir.AluOpType.max)
                            nc.scalar.activation(gs[:, :, 4:5], a2[:],
                                                 mybir.ActivationFunctionType.Exp)
                            ah = gs[:, :, 4:5].to_broadcast([128, KC, 4])
                            nc.vector.tensor_tensor(out=gs[:, :, 0:4], in0=gs[:, :, 0:4],
                                                    in1=ah, op=mybir.AluOpType.mult)
                            # bf16 cast for the scatter matmul rhs
                            g5b = al2p.tile([128, KC, 8], BF)
                            nc.vector.tensor_copy(g5b[:, :, 0:5], gs[:, :, 0:5])
                            for t in range(KC):
                                ti = ch * KC + t
                                sel = selsb[:, w, ch, t * 128 : (t + 1) * 128]
                                nc.tensor.matmul(psO[:, 0:5], sel, g5b[:, t, 0:5],
                                                 start=(ti == 0), stop=(ti == K - 1))
                        dpe = wend2p.tile([128, 1], FP)
                        nc.vector.tensor_scalar_add(dpe[:], psO[:, 4:5], 1e-16)
                        dr = wend2p.tile([128, 1], FP)
                        nc.vector.reciprocal(dr[:], dpe[:])
                        ob = wend2p.tile([128, 8], FP)
                        nc.vector.tensor_tensor(out=ob[:, 0:4], in0=psO[:, 0:4],
                                                in1=dr[:].to_broadcast([128, 4]),
                                                op=mybir.AluOpType.mult)
                        if _build_program.use_b2:
                            nc.vector.tensor_tensor(out=ob[:, 0:4], in0=ob[:, 0:4],
                                                    in1=b2rep[:, 0:4],
                                                    op=mybir.AluOpType.add)
                        nc.sync.dma_start(out_d[w * 128 : (w + 1) * 128, :], ob[:, 0:4])
            else:
                with tc.tile_pool(name="dummy", bufs=1) as dp:
                    z = dp.tile([128, 8], FP)
                    nc.vector.tensor_copy(z[:], b2rep[:])
                    for w in range(NB):
                        nc.sync.dma_start(out_d[w * 128 : (w + 1) * 128, :], z[:, 0:4])

    nc.compile()
    return nc


def _balance_windows(dst):
    """Greedy in-degree balancing of nodes into NWIN windows of WIN slots.

    Returns (node_win, node_slot): window id and slot of every node.
    """
    import heapq

    indeg = np.bincount(dst, minlength=N)
    order = np.argsort(-indeg, kind="stable")
    heap = [(0, w) for w in range(NWIN)]
    heapq.heapify(heap)
    fill = np.zeros(NWIN, np.int64)
    node_win = np.zeros(N, np.int64)
    node_slot = np.zeros(N, np.int64)
    for n in order:
        cnt, w = heapq.heappop(heap)
        node_win[n] = w
        node_slot[n] = fill[w]
        fill[w] += 1
        if fill[w] < WIN:
            heapq.heappush(heap, (cnt + int(indeg[n]), w))
    return node_win, node_slot


def _prepare(x, edge_index, edge_weight, W1, att_src1, att_dst1, att_edge1, We1, b1,
             W2, att_src2, att_dst2, att_edge2, We2, b2):
    x = np.asarray(x, np.float32)
    ei = np.asarray(edge_index)
    ew = np.asarray(edge_weight, np.float32)
    W1 = np.asarray(W1, np.float32)
    att_src1 = np.asarray(att_src1, np.float32)
    att_dst1 = np.asarray(att_dst1, np.float32)
    att_edge1 = np.asarray(att_edge1, np.float32)
    We1 = np.asarray(We1, np.float32)
    b1 = np.asarray(b1, np.float32)
    W2 = np.asarray(W2, np.float32)
    att_src2 = np.asarray(att_src2, np.float32)
    att_dst2 = np.asarray(att_dst2, np.float32)
    att_edge2 = np.asarray(att_edge2, np.float32)
    We2 = np.asarray(We2, np.float32)
    b2 = np.asarray(b2, np.float32)

    # ---------- weight folding (host, weights only) ----------
    W1r = W1.reshape(IN, H, HID)
    Wsrc = np.einsum("khc,hc->kh", W1r, att_src1)
    Wdst = np.einsum("khc,hc->kh", W1r, att_dst1)
    Wsd = np.concatenate(
        [Wsrc, Wdst, np.zeros((IN, TAW - 2 * H), np.float32)], axis=1
    ).astype(np.float32)
    c1 = (We1.reshape(H, HID) * att_edge1).sum(1).astype(np.float32)  # [H]

    # head-minor column order for layer-1 features: new col c*8+h = old h*64+c
    cols = np.tile(np.arange(H), HID) * HID + np.repeat(np.arange(HID), H)
    W1p = np.ascontiguousarray(W1[:, cols])
    b1p = b1[cols]
    W2p = W2[cols, :]

    Waug2 = np.zeros((C1, 8), np.float32)
    Waug2[:, 0:4] = W2p
    Waug2[:, 4] = W2p @ att_src2[0]
    Waug2[:, 5] = W2p @ att_dst2[0]
    W2resh = np.ascontiguousarray(Waug2.reshape(4, 128, 8).transpose(1, 0, 2))
    c2 = float((We2[0] * att_edge2[0]).sum())
    _build_program.c2_host = c2
    _build_program.use_b1 = bool(np.any(b1))
    _build_program.use_b2 = bool(np.any(b2))

    # ---------- edge partitioning (host, index work only) ----------
    src = np.asarray(ei[0], np.int64)
    dst = np.asarray(ei[1], np.int64)

    node_win, node_slot = _balance_windows(dst)
    node_core = node_win // NB
    node_w = node_win % NB
    node_local = node_w * WIN + node_slot          # row in per-core padded space
    node_gpad = node_core * NPAD + node_local      # row in global padded space

    ekey = node_win[dst]
    order = np.argsort(ekey, kind="stable")
    s_s, d_s, w_s = src[order], dst[order], ew[order]
    core_of = node_core[d_s]
    win_of = node_w[d_s]
    loc_of = node_slot[d_s]

    cnt = np.bincount(node_win[d_s], minlength=NWIN)
    K = int(np.ceil(cnt.max() / 128.0))
    NCHo = os.environ.get("BASS_GAT_NCH")
    if NCHo is not None:
        NCH = int(NCHo)
        KC = (K + NCH - 1) // NCH
    else:
        NCH = 2
        while ((K + NCH - 1) // NCH) * 128 > MAXI:
            NCH += 1
        KC = (K + NCH - 1) // NCH
    K = KC * NCH
    SL = KC * 128
    SW = K * 128  # slots per window

    in_maps = []
    base_rep = {
        "xTf": np.ascontiguousarray(x.T),
        "xTb": np.ascontiguousarray(x.T).astype(ml_dtypes.bfloat16),
        "W1b": W1p.astype(ml_dtypes.bfloat16),
        "Wsd": Wsd,
        "W2r": W2resh,
        "c1rep": np.tile(c1[None, :], (128, 1)),
        "b1rep": np.tile(b1p[None, :], (128, 1)),
        "b2rep": np.tile(np.concatenate([b2, np.zeros(4, np.float32)])[None, :], (128, 1)),
        "ident": np.eye(128, dtype=np.float32),
    }

    for c in range(NCORES):
        m = dict(base_rep)
        srcg = np.zeros((NB, NCH, 128, KC * 8), np.int16)
        dstg = np.zeros((NB, NCH, 128, KC * 8), np.int16)
        srcg2 = np.zeros((NB, NCH, 128, KC * 8), np.int16)
        dstg2 = np.zeros((NB, NCH, 128, KC * 8), np.int16)
        ews = np.zeros((NB, NCH, KC, 128), np.float32)
        selT = np.zeros((NB, NCH, 128, SL), np.float32)
        sel_c = core_of == c
        for w in range(NB):
            es = np.nonzero(sel_c & (win_of == w))[0]
            ns = len(es)
            ssrc = np.zeros(SW, np.int64)
            sdst = np.zeros(SW, np.int64)
            sew = np.zeros(SW, np.float32)
            sloc = np.full(SW, -1, np.int64)
            ssrc[:ns] = s_s[es]
            sdst[:ns] = d_s[es]
            sew[:ns] = w_s[es]
            sloc[:ns] = loc_of[es]
            for ch in range(NCH):
                sl = slice(ch * SL, (ch + 1) * SL)
                srcg[w, ch] = _wrap_idx(ssrc[sl])
                dstg[w, ch] = _wrap_idx(sdst[sl])
                srcg2[w, ch] = _wrap_idx(node_gpad[ssrc[sl]])
                dstg2[w, ch] = _wrap_idx(node_local[sdst[sl]])
                ews[w, ch] = sew[sl].reshape(KC, 128)
                lc = sloc[sl]
                valid = np.nonzero(lc >= 0)[0]
                tt, pp = valid // 128, valid % 128
                selT[w, ch, pp, tt * 128 + lc[valid]] = 1.0
        m["srcg"] = np.ascontiguousarray(srcg.transpose(2, 0, 1, 3))
        m["dstg"] = np.ascontiguousarray(dstg.transpose(2, 0, 1, 3))
        m["srcg2"] = np.ascontiguousarray(srcg2.transpose(2, 0, 1, 3))
        m["dstg2"] = np.ascontiguousarray(dstg2.transpose(2, 0, 1, 3))
        m["ew"] = np.ascontiguousarray(ews.transpose(3, 0, 1, 2))
        m["selT"] = np.ascontiguousarray(
            selT.transpose(2, 0, 1, 3)).astype(ml_dtypes.bfloat16)
        in_maps.append(m)

    meta = (node_core, node_local)
    return in_maps, KC, NCH, c2, meta


def kernel(**inputs):
    global LAST_EXEC_NS, LAST_RESULTS
    in_maps, KC, NCH, c2, meta = _prepare(**inputs)
    key = (KC, NCH, c2, _build_program.use_b1, _build_program.use_b2)
    if key not in _CACHE:
        _CACHE[key] = _build_program(KC, NCH)
    nc = _CACHE[key]

    trace = os.environ.get("BASS_GAT_TRACE", "0") == "1"
    res = run_bass_kernel_spmd(nc, in_maps, list(range(NCORES)), trace=trace)
    LAST_EXEC_NS = res.exec_time_ns
    LAST_RESULTS = res
    node_core, node_local = meta
    per_core = [res.results[c]["out_own"] for c in range(NCORES)]
    out = np.empty((N, 4), np.float32)
    for c in range(NCORES):
        mask = node_core == c
        out[mask] = per_core[c][node_local[mask]]
    return out

